# revision 15
# baseline (speedup 1.0000x reference)
"""Multi-head causal attention (B=4, S=2048, D=1024, H=16) on 8 TRN2 NeuronCores.

Sharding: core c -> (batch c//2, head-group c%2 of 8 heads = 512 d_model cols).
Each core:
  - projects Q/K/V for its head slice (bf16 matmuls, fp32 accum)
  - causal attention for its 8 heads over the full sequence, computed with
    scores transposed ([keys, q]) so exp(scores)^T feeds the A@V matmul as the
    moving operand; V is augmented with a ones column so softmax sums fall out
    of the same matmul
  - partial out-projection ctx^T @ Wo[rows-of-its-heads]  (no bias)
Host: out[b] = partial[2b] + partial[2b+1] + bo.
"""

import numpy as np
import ml_dtypes
from contextlib import ExitStack

import concourse.bass as bass
import concourse.tile as tile
from concourse import bacc, mybir
from concourse.bass_utils import run_bass_kernel_spmd

F32 = mybir.dt.float32
BF16 = mybir.dt.bfloat16
EXP = mybir.ActivationFunctionType.Exp

N_CORES = 8
S = 2048          # sequence length
D = 1024          # d_model
HL = 8            # heads per core
HD = 64           # head dim
DL = HL * HD      # local d_model slice = 512
SCALE = 1.0 / 8.0  # 1/sqrt(HD)

NQT = S // 128    # 16 q/seq tiles of 128
NQC = S // 512    # 4 q chunks of 512
NKT = S // 128    # 16 key tiles of 128
NDT = D // 128    # 8 d_model(in) tiles
NMT = DL // 128   # 4 local dout tiles (head pairs)
G = 2             # key-tiles per scores/exp group (2 PSUM banks)

_compiled = None  # cached (nc,) so repeated kernel() calls skip rebuild


def _build():
    nc = bacc.Bacc("TRN2", target_bir_lowering=False, debug=False,
                   num_devices=N_CORES)

    xq_ap = nc.dram_tensor("xqt", [D, S], BF16, kind="ExternalInput").ap()
    xk_ap = nc.dram_tensor("xkt", [D, S], BF16, kind="ExternalInput").ap()
    xv_ap = nc.dram_tensor("xvt", [D, S], BF16, kind="ExternalInput").ap()
    wq_ap = nc.dram_tensor("wq", [D, DL], BF16, kind="ExternalInput").ap()
    wk_ap = nc.dram_tensor("wk", [D, DL], BF16, kind="ExternalInput").ap()
    wv_ap = nc.dram_tensor("wv", [D, DL], BF16, kind="ExternalInput").ap()
    bq_ap = nc.dram_tensor("bq", [DL, 1], F32, kind="ExternalInput").ap()
    bk_ap = nc.dram_tensor("bk", [DL, 1], F32, kind="ExternalInput").ap()
    bvb_ap = nc.dram_tensor("bvb", [128, DL], F32, kind="ExternalInput").ap()
    wo_ap = nc.dram_tensor("wo", [DL, D], BF16, kind="ExternalInput").ap()
    out_ap = nc.dram_tensor("out", [S, D], F32, kind="ExternalOutput").ap()

    with tile.TileContext(nc) as tc, ExitStack() as ctx:
        wpool = ctx.enter_context(tc.tile_pool(name="weights", bufs=1))
        xt_pool = ctx.enter_context(tc.tile_pool(name="xt", bufs=16))
        qkv_pool = ctx.enter_context(tc.tile_pool(name="qkv", bufs=1))
        exp_pool = ctx.enter_context(tc.tile_pool(name="expt", bufs=3))
        norm_pool = ctx.enter_context(tc.tile_pool(name="norm", bufs=4))
        outst_pool = ctx.enter_context(tc.tile_pool(name="outst", bufs=2))
        psum_big = ctx.enter_context(tc.tile_pool(name="ps_big", bufs=2, space="PSUM"))
        psum_ctx = ctx.enter_context(tc.tile_pool(name="ps_ctx", bufs=2, space="PSUM"))
        psum_work = ctx.enter_context(tc.tile_pool(name="ps_work", bufs=2, space="PSUM"))

        # ---- weights / biases (already bf16 in DRAM) ----
        def load_w(dram, shape, nm):
            t16 = wpool.tile(shape, BF16, tag=nm, name=nm)
            nc.sync.dma_start(t16[:], dram)
            return t16

        wq_sb = [load_w(wq_ap[128 * d:128 * (d + 1), :], [128, DL], f"wq{d}") for d in range(NDT)]
        wk_sb = [load_w(wk_ap[128 * d:128 * (d + 1), :], [128, DL], f"wk{d}") for d in range(NDT)]
        wv_sb = [load_w(wv_ap[128 * d:128 * (d + 1), :], [128, DL], f"wv{d}") for d in range(NDT)]
        wo_sb = [load_w(wo_ap[128 * d:128 * (d + 1), :], [128, D], f"wo{d}") for d in range(NMT)]

        bq_sb = wpool.tile([128, NMT], F32, tag="bq")
        bk_sb = wpool.tile([128, NMT], F32, tag="bk")
        for m in range(NMT):
            nc.sync.dma_start(bq_sb[:, m:m + 1], bq_ap[128 * m:128 * (m + 1), :])
            nc.sync.dma_start(bk_sb[:, m:m + 1], bk_ap[128 * m:128 * (m + 1), :])
        bvb_sb = wpool.tile([128, DL], F32, tag="bvb")
        nc.sync.dma_start(bvb_sb[:], bvb_ap[:])

        # ---- x^T load (host pre-transposed + pre-cast bf16) ----
        # full-row tiles [128 din, S]
        def load_xt(x_ap, nm):
            xt = []
            for d in range(NDT):
                t = xt_pool.tile([128, S], BF16, tag="xt", name=f"{nm}xt{d}")
                nc.sync.dma_start(t[:], x_ap[128 * d:128 * (d + 1), :])
                xt.append(t)
            return xt

        # ---- projections ----
        # qT/kT: [DL, S] bf16 stored as NMT tiles [128, S]
        def proj_T(xt, w_sb, b_sb, name):
            res = [qkv_pool.tile([128, S], BF16, tag=f"{name}{m}", name=f"{name}{m}") for m in range(NMT)]
            for qc in range(NQC):
                for m in range(NMT):
                    ps = psum_work.tile([128, 512], F32, tag="work", name="ps")
                    for d in range(NDT):
                        nc.tensor.matmul(
                            ps[:], w_sb[d][:, 128 * m:128 * (m + 1)],
                            xt[d][:, 512 * qc:512 * (qc + 1)],
                            start=(d == 0), stop=(d == NDT - 1))
                    nc.vector.tensor_scalar_add(
                        res[m][:, 512 * qc:512 * (qc + 1)], ps[:],
                        b_sb[:, m:m + 1])
            return res

        xqt = load_xt(xq_ap, "q")
        qT = proj_T(xqt, wq_sb, bq_sb, "qT")
        xkt = load_xt(xk_ap, "k")
        kT = proj_T(xkt, wk_sb, bk_sb, "kT")

        # v_aug: per seq-tile [128, HL*(HD+1)] bf16; per head 64 v cols + ones col
        v_aug = []
        xvt = load_xt(xv_ap, "v")
        for st in range(NQT):
                va = qkv_pool.tile([128, HL * (HD + 1)], BF16, tag=f"va{st}")
                nc.vector.memset(va[:], 1.0)
                ps = psum_work.tile([128, DL], F32, tag="work", name="ps")
                for d in range(NDT):
                    nc.tensor.matmul(ps[:], xvt[d][:, 128 * st:128 * (st + 1)],
                                     wv_sb[d][:], start=(d == 0), stop=(d == NDT - 1))
                va3 = va[:].rearrange("p (h c) -> p h c", h=HL)[:, :, 0:HD]
                nc.vector.tensor_add(
                    va3,
                    ps[:].rearrange("p (h c) -> p h c", h=HL),
                    bvb_sb[:].rearrange("p (h c) -> p h c", h=HL))
                v_aug.append(va)

        # ---- attention + out projection, per q-chunk ----
        # ctxT: per head-pair tile [128, S] bf16 (rows 64*(h%2) for head h)
        ctxT = [qkv_pool.tile([128, S], BF16, tag=f"ctxT{m}", name=f"ctxT{m}") for m in range(NMT)]

        for qc in range(NQC):
            nkt = 4 * (qc + 1)  # causal: key tiles 0..nkt-1
            for hp in range(HL // 2):
                heads = (2 * hp, 2 * hp + 1)
                m = hp  # both heads live in kT/qT tile m=h//2=hp
                ctx_ps = {h: psum_ctx.tile([HD + 1, 512], F32, tag="ctx",
                                           name=f"ctx{h}") for h in heads}
                items = [(h, kt) for kt in range(nkt) for h in heads]
                for g0 in range(0, len(items), G):
                    grp = items[g0:g0 + G]
                    sc_ps = psum_big.tile([128, 512 * G], F32, tag="big", name="sc")
                    for i, (h, kt) in enumerate(grp):
                        po = 64 * (h % 2)
                        qs = max(0, 128 * kt - 512 * qc)  # local q start
                        nc.tensor.matmul(
                            sc_ps[:, 512 * i + qs:512 * (i + 1)],
                            kT[m][po:po + HD, 128 * kt:128 * (kt + 1)],
                            qT[m][po:po + HD, 512 * qc + qs:512 * (qc + 1)],
                            start=True, stop=True)
                    et = exp_pool.tile([128, 512 * G], BF16, tag="et", name="et")
                    nc.scalar.activation(et[:, :512 * len(grp)],
                                         sc_ps[:, :512 * len(grp)],
                                         EXP, scale=SCALE)
                    for i, (h, kt) in enumerate(grp):
                        qs = max(0, 128 * kt - 512 * qc)
                        if 4 * qc <= kt < 4 * qc + 4:  # diagonal block: mask k>q
                            nc.gpsimd.affine_select(
                                out=et[:, 512 * i + qs:512 * i + qs + 128],
                                in_=et[:, 512 * i + qs:512 * i + qs + 128],
                                compare_op=mybir.AluOpType.is_ge, fill=0.0,
                                base=0, pattern=[[1, 128]], channel_multiplier=-1)
                        nc.tensor.matmul(
                            ctx_ps[h][:, qs:512],
                            v_aug[kt][:].rearrange("p (h c) -> p h c", h=HL)[:, h, :],
                            et[:, 512 * i + qs:512 * (i + 1)],
                            start=(kt == 0), stop=(kt == nkt - 1))
                        if kt == nkt - 1:
                            # head h complete: normalize into ctxT
                            po = 64 * (h % 2)
                            sums_sb = norm_pool.tile([1, 512], F32, tag="sums",
                                                     name="sums")
                            nc.vector.tensor_copy(sums_sb[:],
                                                  ctx_ps[h][HD:HD + 1, :])
                            recip = norm_pool.tile([1, 512], F32, tag="recip",
                                                   name="recip")
                            nc.vector.reciprocal_approx_fast(recip[:], sums_sb[:])
                            rep = norm_pool.tile([HD, 512], F32, tag="rep",
                                                 name="rep")
                            nc.sync.dma_start(
                                rep[:],
                                recip[:].unsqueeze(1).broadcast_to([1, HD, 512]))
                            nc.vector.tensor_mul(
                                ctxT[m][po:po + HD, 512 * qc:512 * (qc + 1)],
                                ctx_ps[h][0:HD, :], rep[:])

            # out-projection for this q chunk (4 q-tiles of 128)
            for qt in range(4 * qc, 4 * qc + 4):
                ot = outst_pool.tile([128, 1024], F32, tag="ot")
                for n in range(2):
                    po_ps = psum_work.tile([128, 512], F32, tag="work", name="po_ps")
                    for d in range(NMT):
                        nc.tensor.matmul(
                            po_ps[:],
                            ctxT[d][:, 128 * qt:128 * (qt + 1)],
                            wo_sb[d][:, 512 * n:512 * (n + 1)],
                            start=(d == 0), stop=(d == NMT - 1))
                    nc.vector.tensor_copy(ot[:, 512 * n:512 * (n + 1)], po_ps[:])
                nc.sync.dma_start(out_ap[128 * qt:128 * (qt + 1), :], ot[:])

    nc.compile()
    return nc


def _shard(inputs):
    in_maps = []
    for c in range(N_CORES):
        b, g = c // 2, c % 2
        sl = slice(512 * g, 512 * (g + 1))
        in_maps.append({
            "xqt": np.ascontiguousarray(inputs["inputs_q"][b].T.astype(ml_dtypes.bfloat16)),
            "xkt": np.ascontiguousarray(inputs["inputs_k"][b].T.astype(ml_dtypes.bfloat16)),
            "xvt": np.ascontiguousarray(inputs["inputs_v"][b].T.astype(ml_dtypes.bfloat16)),
            "wq": np.ascontiguousarray(inputs["Wq"][:, sl].astype(ml_dtypes.bfloat16)),
            "wk": np.ascontiguousarray(inputs["Wk"][:, sl].astype(ml_dtypes.bfloat16)),
            "wv": np.ascontiguousarray(inputs["Wv"][:, sl].astype(ml_dtypes.bfloat16)),
            "bq": np.ascontiguousarray(inputs["bq"][sl])[:, None],
            "bk": np.ascontiguousarray(inputs["bk"][sl])[:, None],
            "bvb": np.ascontiguousarray(
                np.broadcast_to(inputs["bv"][sl], (128, 512))),
            "wo": np.ascontiguousarray(inputs["Wo"][sl, :].astype(ml_dtypes.bfloat16)),
        })
    return in_maps


def kernel(**inputs):
    global _compiled
    inputs = {k: np.asarray(v, dtype=np.float32) for k, v in inputs.items()}
    if _compiled is None:
        _compiled = _build()
    nc = _compiled
    in_maps = _shard(inputs)
    res = run_bass_kernel_spmd(nc, in_maps, list(range(N_CORES)),
                               trace=bool(int(__import__("os").environ.get("BASS_TRACE", "0"))))
    kernel.last_results = res
    B = 4
    out = np.empty((B, S, D), np.float32)
    for b in range(B):
        out[b] = res.results[2 * b]["out"] + res.results[2 * b + 1]["out"]
    out += inputs["bo"][None, None, :]
    return out


# revision 16
# speedup vs baseline: 1.3920x; 1.3920x over previous
"""Multi-head causal attention (B=4, S=2048, D=1024, H=16) on 8 TRN2 NeuronCores.

Sharding: core c -> (batch c//2, head-group c%2 of 8 heads = 512 d_model cols).
Each core:
  - projects Q/K/V for its head slice (bf16 matmuls, fp32 accum)
  - causal attention for its 8 heads over the full sequence, computed with
    scores transposed ([keys, q]) so exp(scores)^T feeds the A@V matmul as the
    moving operand; V is augmented with a ones column so softmax sums fall out
    of the same matmul
  - partial out-projection ctx^T @ Wo[rows-of-its-heads]  (no bias)
Host: out[b] = partial[2b] + partial[2b+1] + bo.
"""

import numpy as np
import ml_dtypes
from contextlib import ExitStack

import concourse.bass as bass
import concourse.tile as tile
from concourse import bacc, mybir
from concourse.bass_utils import run_bass_kernel_spmd

F32 = mybir.dt.float32
BF16 = mybir.dt.bfloat16
EXP = mybir.ActivationFunctionType.Exp

N_CORES = 8
S = 2048          # sequence length
D = 1024          # d_model
HL = 8            # heads per core
HD = 64           # head dim
DL = HL * HD      # local d_model slice = 512
SCALE = 1.0 / 8.0  # 1/sqrt(HD)

NQT = S // 128    # 16 q/seq tiles of 128
NQC = S // 512    # 4 q chunks of 512
NKT = S // 128    # 16 key tiles of 128
NDT = D // 128    # 8 d_model(in) tiles
NMT = DL // 128   # 4 local dout tiles (head pairs)
G = 2             # key-tiles per scores/exp group (2 PSUM banks)

_compiled = None  # cached (nc,) so repeated kernel() calls skip rebuild


def _build():
    nc = bacc.Bacc("TRN2", target_bir_lowering=False, debug=False,
                   num_devices=N_CORES)

    xq_ap = nc.dram_tensor("xqt", [D, S], BF16, kind="ExternalInput").ap()
    xk_ap = nc.dram_tensor("xkt", [D, S], BF16, kind="ExternalInput").ap()
    xv_ap = nc.dram_tensor("xvt", [D, S], BF16, kind="ExternalInput").ap()
    wq_ap = nc.dram_tensor("wq", [D, DL], BF16, kind="ExternalInput").ap()
    wk_ap = nc.dram_tensor("wk", [D, DL], BF16, kind="ExternalInput").ap()
    wv_ap = nc.dram_tensor("wv", [D, DL], BF16, kind="ExternalInput").ap()
    bq_ap = nc.dram_tensor("bq", [DL, 1], F32, kind="ExternalInput").ap()
    bk_ap = nc.dram_tensor("bk", [DL, 1], F32, kind="ExternalInput").ap()
    bvb_ap = nc.dram_tensor("bvb", [128, DL], F32, kind="ExternalInput").ap()
    wo_ap = nc.dram_tensor("wo", [DL, D], BF16, kind="ExternalInput").ap()
    out_ap = nc.dram_tensor("out", [S, D], F32, kind="ExternalOutput").ap()

    with tile.TileContext(nc) as tc, ExitStack() as ctx:
        wpool = ctx.enter_context(tc.tile_pool(name="weights", bufs=1))
        xt_pool = ctx.enter_context(tc.tile_pool(name="xt", bufs=16))
        qkv_pool = ctx.enter_context(tc.tile_pool(name="qkv", bufs=1))
        exp_pool = ctx.enter_context(tc.tile_pool(name="expt", bufs=3))
        norm_pool = ctx.enter_context(tc.tile_pool(name="norm", bufs=4))
        outst_pool = ctx.enter_context(tc.tile_pool(name="outst", bufs=2))
        psum_big = ctx.enter_context(tc.tile_pool(name="ps_big", bufs=2, space="PSUM"))
        psum_ctx = ctx.enter_context(tc.tile_pool(name="ps_ctx", bufs=4, space="PSUM"))

        # ---- weights / biases (already bf16 in DRAM) ----
        def load_w(dram, shape, nm):
            t16 = wpool.tile(shape, BF16, tag=nm, name=nm)
            nc.sync.dma_start(t16[:], dram)
            return t16

        wq_sb = [load_w(wq_ap[128 * d:128 * (d + 1), :], [128, DL], f"wq{d}") for d in range(NDT)]
        wk_sb = [load_w(wk_ap[128 * d:128 * (d + 1), :], [128, DL], f"wk{d}") for d in range(NDT)]
        wv_sb = [load_w(wv_ap[128 * d:128 * (d + 1), :], [128, DL], f"wv{d}") for d in range(NDT)]
        wo_sb = [load_w(wo_ap[128 * d:128 * (d + 1), :], [128, D], f"wo{d}") for d in range(NMT)]

        bq_sb = wpool.tile([128, NMT], F32, tag="bq")
        bk_sb = wpool.tile([128, NMT], F32, tag="bk")
        for m in range(NMT):
            nc.sync.dma_start(bq_sb[:, m:m + 1], bq_ap[128 * m:128 * (m + 1), :])
            nc.sync.dma_start(bk_sb[:, m:m + 1], bk_ap[128 * m:128 * (m + 1), :])
        bvb_sb = wpool.tile([128, DL], F32, tag="bvb")
        nc.sync.dma_start(bvb_sb[:], bvb_ap[:])

        # ---- x^T load (host pre-transposed + pre-cast bf16) ----
        # full-row tiles [128 din, S]
        def load_xt(x_ap, nm):
            xt = []
            for d in range(NDT):
                t = xt_pool.tile([128, S], BF16, tag="xt", name=f"{nm}xt{d}")
                nc.sync.dma_start(t[:], x_ap[128 * d:128 * (d + 1), :])
                xt.append(t)
            return xt

        # ---- projections ----
        # qT/kT: [DL, S] bf16 stored as NMT tiles [128, S]
        def proj_T(xt, w_sb, b_sb, name):
            res = [qkv_pool.tile([128, S], BF16, tag=f"{name}{m}", name=f"{name}{m}") for m in range(NMT)]
            for qc in range(NQC):
                for m in range(NMT):
                    ps = psum_big.tile([128, 512], F32, tag="big", name="ps")
                    for d in range(NDT):
                        nc.tensor.matmul(
                            ps[:], w_sb[d][:, 128 * m:128 * (m + 1)],
                            xt[d][:, 512 * qc:512 * (qc + 1)],
                            start=(d == 0), stop=(d == NDT - 1))
                    nc.vector.tensor_scalar_add(
                        res[m][:, 512 * qc:512 * (qc + 1)], ps[:],
                        b_sb[:, m:m + 1])
            return res

        xqt = load_xt(xq_ap, "q")
        qT = proj_T(xqt, wq_sb, bq_sb, "qT")
        xkt = load_xt(xk_ap, "k")
        kT = proj_T(xkt, wk_sb, bk_sb, "kT")

        # v_aug: per seq-tile [128, HL*(HD+1)] bf16; per head 64 v cols + ones col
        v_aug = []
        xvt = load_xt(xv_ap, "v")
        for st in range(NQT):
                va = qkv_pool.tile([128, HL * (HD + 1)], BF16, tag=f"va{st}")
                nc.vector.memset(va[:], 1.0)
                ps = psum_big.tile([128, DL], F32, tag="big", name="ps")
                for d in range(NDT):
                    nc.tensor.matmul(ps[:], xvt[d][:, 128 * st:128 * (st + 1)],
                                     wv_sb[d][:], start=(d == 0), stop=(d == NDT - 1))
                va3 = va[:].rearrange("p (h c) -> p h c", h=HL)[:, :, 0:HD]
                nc.vector.tensor_add(
                    va3,
                    ps[:].rearrange("p (h c) -> p h c", h=HL),
                    bvb_sb[:].rearrange("p (h c) -> p h c", h=HL))
                v_aug.append(va)

        # ---- attention + out projection, per q-chunk ----
        # ctxT: per head-pair tile [128, S] bf16 (rows 64*(h%2) for head h)
        ctxT = [qkv_pool.tile([128, S], BF16, tag=f"ctxT{m}", name=f"ctxT{m}") for m in range(NMT)]

        def emit_outproj(qt):
            ot = outst_pool.tile([128, 1024], F32, tag="ot", name="ot")
            for n in range(2):
                po_ps = psum_big.tile([128, 512], F32, tag="big", name="po_ps")
                for d in range(NMT):
                    nc.tensor.matmul(
                        po_ps[:],
                        ctxT[d][:, 128 * qt:128 * (qt + 1)],
                        wo_sb[d][:, 512 * n:512 * (n + 1)],
                        start=(d == 0), stop=(d == NMT - 1))
                nc.vector.tensor_copy(ot[:, 512 * n:512 * (n + 1)], po_ps[:])
            nc.sync.dma_start(out_ap[128 * qt:128 * (qt + 1), :], ot[:])

        for qc in range(NQC):
            nkt = 4 * (qc + 1)  # causal: key tiles 0..nkt-1
            for h in range(HL):
                m, po = h // 2, 64 * (h % 2)
                ctx_ps = psum_ctx.tile([HD + 1, 512], F32, tag="ctx",
                                       name=f"ctx{h}")
                for g0 in range(0, nkt, G):
                    gn = min(G, nkt - g0)
                    sc_ps = psum_big.tile([128, 512 * G], F32, tag="big", name="sc")
                    for i in range(gn):
                        kt = g0 + i
                        qs = max(0, 128 * kt - 512 * qc)  # local q start
                        nc.tensor.matmul(
                            sc_ps[:, 512 * i + qs:512 * (i + 1)],
                            kT[m][po:po + HD, 128 * kt:128 * (kt + 1)],
                            qT[m][po:po + HD, 512 * qc + qs:512 * (qc + 1)],
                            start=True, stop=True)
                    et = exp_pool.tile([128, 512 * G], BF16, tag="et", name="et")
                    nc.scalar.activation(et[:, :512 * gn], sc_ps[:, :512 * gn],
                                         EXP, scale=SCALE)
                    for i in range(gn):
                        kt = g0 + i
                        qs = max(0, 128 * kt - 512 * qc)
                        if 4 * qc <= kt < 4 * qc + 4:  # diagonal block: mask k>q
                            nc.gpsimd.affine_select(
                                out=et[:, 512 * i + qs:512 * i + qs + 128],
                                in_=et[:, 512 * i + qs:512 * i + qs + 128],
                                compare_op=mybir.AluOpType.is_ge, fill=0.0,
                                base=0, pattern=[[1, 128]], channel_multiplier=-1)
                        nc.tensor.matmul(
                            ctx_ps[:, qs:512],
                            v_aug[kt][:].rearrange("p (h c) -> p h c", h=HL)[:, h, :],
                            et[:, 512 * i + qs:512 * (i + 1)],
                            start=(kt == 0), stop=(kt == nkt - 1))
                # normalize into ctxT
                sums_sb = norm_pool.tile([1, 512], F32, tag="sums", name="sums")
                nc.vector.tensor_copy(sums_sb[:], ctx_ps[HD:HD + 1, :])
                recip = norm_pool.tile([1, 512], F32, tag="recip", name="recip")
                nc.vector.reciprocal_approx_fast(recip[:], sums_sb[:])
                rep = norm_pool.tile([HD, 512], F32, tag="rep", name="rep")
                nc.gpsimd.partition_broadcast(rep[:], recip[:])
                nc.vector.tensor_mul(
                    ctxT[m][po:po + HD, 512 * qc:512 * (qc + 1)],
                    ctx_ps[0:HD, :], rep[:])
                # sprinkle previous chunk's out-projection between heads
                if qc > 0 and h % 2 == 1:
                    emit_outproj(4 * (qc - 1) + h // 2)

        for qt in range(4 * (NQC - 1), 4 * NQC):
            emit_outproj(qt)

    nc.compile()
    return nc


def _shard(inputs):
    in_maps = []
    for c in range(N_CORES):
        b, g = c // 2, c % 2
        sl = slice(512 * g, 512 * (g + 1))
        in_maps.append({
            "xqt": np.ascontiguousarray(inputs["inputs_q"][b].T.astype(ml_dtypes.bfloat16)),
            "xkt": np.ascontiguousarray(inputs["inputs_k"][b].T.astype(ml_dtypes.bfloat16)),
            "xvt": np.ascontiguousarray(inputs["inputs_v"][b].T.astype(ml_dtypes.bfloat16)),
            "wq": np.ascontiguousarray(inputs["Wq"][:, sl].astype(ml_dtypes.bfloat16)),
            "wk": np.ascontiguousarray(inputs["Wk"][:, sl].astype(ml_dtypes.bfloat16)),
            "wv": np.ascontiguousarray(inputs["Wv"][:, sl].astype(ml_dtypes.bfloat16)),
            "bq": np.ascontiguousarray(inputs["bq"][sl])[:, None],
            "bk": np.ascontiguousarray(inputs["bk"][sl])[:, None],
            "bvb": np.ascontiguousarray(
                np.broadcast_to(inputs["bv"][sl], (128, 512))),
            "wo": np.ascontiguousarray(inputs["Wo"][sl, :].astype(ml_dtypes.bfloat16)),
        })
    return in_maps


def kernel(**inputs):
    global _compiled
    inputs = {k: np.asarray(v, dtype=np.float32) for k, v in inputs.items()}
    if _compiled is None:
        _compiled = _build()
    nc = _compiled
    in_maps = _shard(inputs)
    res = run_bass_kernel_spmd(nc, in_maps, list(range(N_CORES)),
                               trace=bool(int(__import__("os").environ.get("BASS_TRACE", "0"))))
    kernel.last_results = res
    B = 4
    out = np.empty((B, S, D), np.float32)
    for b in range(B):
        out[b] = res.results[2 * b]["out"] + res.results[2 * b + 1]["out"]
    out += inputs["bo"][None, None, :]
    return out


# revision 17
# speedup vs baseline: 1.6054x; 1.1533x over previous
"""Multi-head causal attention (B=4, S=2048, D=1024, H=16) on 8 TRN2 NeuronCores.

Sharding: core c -> (batch c//2, head-group c%2 of 8 heads = 512 d_model cols).
Each core:
  - projects Q/K/V for its head slice (bf16 matmuls, fp32 accum)
  - causal attention for its 8 heads over the full sequence, computed with
    scores transposed ([keys, q]) so exp(scores)^T feeds the A@V matmul as the
    moving operand; V is augmented with a ones column so softmax sums fall out
    of the same matmul
  - partial out-projection ctx^T @ Wo[rows-of-its-heads]  (no bias)
Host: out[b] = partial[2b] + partial[2b+1] + bo.
"""

import numpy as np
import ml_dtypes
from contextlib import ExitStack

import concourse.bass as bass
import concourse.tile as tile
from concourse import bacc, mybir
from concourse.bass_utils import run_bass_kernel_spmd

F32 = mybir.dt.float32
BF16 = mybir.dt.bfloat16
EXP = mybir.ActivationFunctionType.Exp

N_CORES = 8
S = 2048          # sequence length
D = 1024          # d_model
HL = 8            # heads per core
HD = 64           # head dim
DL = HL * HD      # local d_model slice = 512
SCALE = 1.0 / 8.0  # 1/sqrt(HD)

NQT = S // 128    # 16 q/seq tiles of 128
NQC = S // 512    # 4 q chunks of 512
NKT = S // 128    # 16 key tiles of 128
NDT = D // 128    # 8 d_model(in) tiles
NMT = DL // 128   # 4 local dout tiles (head pairs)
G = 2             # key-tiles per scores/exp group (2 PSUM banks)

_compiled = None  # cached (nc,) so repeated kernel() calls skip rebuild


def _build():
    nc = bacc.Bacc("TRN2", target_bir_lowering=False, debug=False,
                   num_devices=N_CORES)

    xq_ap = nc.dram_tensor("xqt", [D, S], BF16, kind="ExternalInput").ap()
    xk_ap = nc.dram_tensor("xkt", [D, S], BF16, kind="ExternalInput").ap()
    xv_ap = nc.dram_tensor("xvt", [D, S], BF16, kind="ExternalInput").ap()
    wq_ap = nc.dram_tensor("wq", [D, DL], BF16, kind="ExternalInput").ap()
    wk_ap = nc.dram_tensor("wk", [D, DL], BF16, kind="ExternalInput").ap()
    wv_ap = nc.dram_tensor("wv", [D, DL], BF16, kind="ExternalInput").ap()
    bq_ap = nc.dram_tensor("bq", [DL, 1], F32, kind="ExternalInput").ap()
    bk_ap = nc.dram_tensor("bk", [DL, 1], F32, kind="ExternalInput").ap()
    bvb_ap = nc.dram_tensor("bvb", [128, DL], F32, kind="ExternalInput").ap()
    wo_ap = nc.dram_tensor("wo", [DL, D], BF16, kind="ExternalInput").ap()
    out_ap = nc.dram_tensor("out", [S, D], F32, kind="ExternalOutput").ap()

    with tile.TileContext(nc) as tc, ExitStack() as ctx:
        wpool = ctx.enter_context(tc.tile_pool(name="weights", bufs=1))
        xt_pool = ctx.enter_context(tc.tile_pool(name="xt", bufs=64))
        qkv_pool = ctx.enter_context(tc.tile_pool(name="qkv", bufs=1))
        exp_pool = ctx.enter_context(tc.tile_pool(name="expt", bufs=4))
        norm_pool = ctx.enter_context(tc.tile_pool(name="norm", bufs=4))
        outst_pool = ctx.enter_context(tc.tile_pool(name="outst", bufs=2))
        psum_big = ctx.enter_context(tc.tile_pool(name="ps_big", bufs=3, space="PSUM"))
        psum_ctx = ctx.enter_context(tc.tile_pool(name="ps_ctx", bufs=2, space="PSUM"))

        # ---- weights / biases (already bf16 in DRAM) ----
        def load_w(dram, shape, nm):
            t16 = wpool.tile(shape, BF16, tag=nm, name=nm)
            nc.sync.dma_start(t16[:], dram)
            return t16

        wq_sb = [load_w(wq_ap[128 * d:128 * (d + 1), :], [128, DL], f"wq{d}") for d in range(NDT)]
        wk_sb = [load_w(wk_ap[128 * d:128 * (d + 1), :], [128, DL], f"wk{d}") for d in range(NDT)]
        wv_sb = [load_w(wv_ap[128 * d:128 * (d + 1), :], [128, DL], f"wv{d}") for d in range(NDT)]
        wo_sb = [load_w(wo_ap[128 * d:128 * (d + 1), :], [128, D], f"wo{d}") for d in range(NMT)]

        bq_sb = wpool.tile([128, NMT], F32, tag="bq")
        bk_sb = wpool.tile([128, NMT], F32, tag="bk")
        for m in range(NMT):
            nc.sync.dma_start(bq_sb[:, m:m + 1], bq_ap[128 * m:128 * (m + 1), :])
            nc.sync.dma_start(bk_sb[:, m:m + 1], bk_ap[128 * m:128 * (m + 1), :])
        bvb_sb = wpool.tile([128, DL], F32, tag="bvb")
        nc.sync.dma_start(bvb_sb[:], bvb_ap[:])

        # ---- x^T chunk load (host pre-transposed + pre-cast bf16) ----
        # 8 tiles [128 din, 512 seq] per (input, chunk)
        def load_xt_chunk(x_ap, qc, nm):
            xt = []
            for d in range(NDT):
                t = xt_pool.tile([128, 512], BF16, tag="xt", name=f"{nm}xt{qc}_{d}")
                nc.sync.dma_start(
                    t[:], x_ap[128 * d:128 * (d + 1), 512 * qc:512 * (qc + 1)])
                xt.append(t)
            return xt

        # qT/kT: [DL, S] bf16 stored as NMT tiles [128, S]
        qT = [qkv_pool.tile([128, S], BF16, tag=f"qT{m}", name=f"qT{m}") for m in range(NMT)]
        kT = [qkv_pool.tile([128, S], BF16, tag=f"kT{m}", name=f"kT{m}") for m in range(NMT)]

        def proj_chunk(xt, w_sb, b_sb, res, qc):
            for m in range(NMT):
                ps = psum_big.tile([128, 512], F32, tag="big", name="ps")
                for d in range(NDT):
                    nc.tensor.matmul(
                        ps[:], w_sb[d][:, 128 * m:128 * (m + 1)],
                        xt[d][:],
                        start=(d == 0), stop=(d == NDT - 1))
                nc.vector.tensor_scalar_add(
                    res[m][:, 512 * qc:512 * (qc + 1)], ps[:],
                    b_sb[:, m:m + 1])

        # v_aug: per seq-tile [128, HL*(HD+1)] bf16; per head 64 v cols + ones col
        v_aug = [None] * NQT

        def v_chunk(xt, qc):
            for sti in range(4):
                st = 4 * qc + sti
                va = qkv_pool.tile([128, HL * (HD + 1)], BF16, tag=f"va{st}",
                                   name=f"va{st}")
                nc.vector.memset(va[:], 1.0)
                ps = psum_big.tile([128, DL], F32, tag="big", name="ps")
                for d in range(NDT):
                    nc.tensor.matmul(ps[:], xt[d][:, 128 * sti:128 * (sti + 1)],
                                     wv_sb[d][:], start=(d == 0), stop=(d == NDT - 1))
                va3 = va[:].rearrange("p (h c) -> p h c", h=HL)[:, :, 0:HD]
                nc.vector.tensor_add(
                    va3,
                    ps[:].rearrange("p (h c) -> p h c", h=HL),
                    bvb_sb[:].rearrange("p (h c) -> p h c", h=HL))
                v_aug[st] = va

        # ---- attention + out projection, per q-chunk ----
        # ctxT: per head-pair tile [128, S] bf16 (rows 64*(h%2) for head h)
        ctxT = [qkv_pool.tile([128, S], BF16, tag=f"ctxT{m}", name=f"ctxT{m}") for m in range(NMT)]

        def emit_outproj(qt):
            ot = outst_pool.tile([128, 1024], F32, tag="ot", name="ot")
            for n in range(2):
                po_ps = psum_big.tile([128, 512], F32, tag="big", name="po_ps")
                for d in range(NMT):
                    nc.tensor.matmul(
                        po_ps[:],
                        ctxT[d][:, 128 * qt:128 * (qt + 1)],
                        wo_sb[d][:, 512 * n:512 * (n + 1)],
                        start=(d == 0), stop=(d == NMT - 1))
                nc.vector.tensor_copy(ot[:, 512 * n:512 * (n + 1)], po_ps[:])
            nc.sync.dma_start(out_ap[128 * qt:128 * (qt + 1), :], ot[:])

        for qc in range(NQC):
            xq_c = load_xt_chunk(xq_ap, qc, "q")
            proj_chunk(xq_c, wq_sb, bq_sb, qT, qc)
            xk_c = load_xt_chunk(xk_ap, qc, "k")
            proj_chunk(xk_c, wk_sb, bk_sb, kT, qc)
            xv_c = load_xt_chunk(xv_ap, qc, "v")
            v_chunk(xv_c, qc)
            nkt = 4 * (qc + 1)  # causal: key tiles 0..nkt-1
            for h in range(HL):
                m, po = h // 2, 64 * (h % 2)
                ctx_ps = psum_ctx.tile([HD + 1, 512], F32, tag="ctx",
                                       name=f"ctx{h}")
                for g0 in range(0, nkt, G):
                    gn = min(G, nkt - g0)
                    sc_ps = psum_big.tile([128, 512 * G], F32, tag="big", name="sc")
                    for i in range(gn):
                        kt = g0 + i
                        qs = max(0, 128 * kt - 512 * qc)  # local q start
                        nc.tensor.matmul(
                            sc_ps[:, 512 * i + qs:512 * (i + 1)],
                            kT[m][po:po + HD, 128 * kt:128 * (kt + 1)],
                            qT[m][po:po + HD, 512 * qc + qs:512 * (qc + 1)],
                            start=True, stop=True)
                    et = exp_pool.tile([128, 512 * G], BF16, tag="et", name="et")
                    nc.scalar.activation(et[:, :512 * gn], sc_ps[:, :512 * gn],
                                         EXP, scale=SCALE)
                    for i in range(gn):
                        kt = g0 + i
                        qs = max(0, 128 * kt - 512 * qc)
                        if 4 * qc <= kt < 4 * qc + 4:  # diagonal block: mask k>q
                            nc.gpsimd.affine_select(
                                out=et[:, 512 * i + qs:512 * i + qs + 128],
                                in_=et[:, 512 * i + qs:512 * i + qs + 128],
                                compare_op=mybir.AluOpType.is_ge, fill=0.0,
                                base=0, pattern=[[1, 128]], channel_multiplier=-1)
                        nc.tensor.matmul(
                            ctx_ps[:, qs:512],
                            v_aug[kt][:].rearrange("p (h c) -> p h c", h=HL)[:, h, :],
                            et[:, 512 * i + qs:512 * (i + 1)],
                            start=(kt == 0), stop=(kt == nkt - 1))
                # normalize into ctxT
                sums_sb = norm_pool.tile([1, 512], F32, tag="sums", name="sums")
                nc.vector.tensor_copy(sums_sb[:], ctx_ps[HD:HD + 1, :])
                recip = norm_pool.tile([1, 512], F32, tag="recip", name="recip")
                nc.vector.reciprocal_approx_fast(recip[:], sums_sb[:])
                rep = norm_pool.tile([HD, 512], F32, tag="rep", name="rep")
                nc.gpsimd.partition_broadcast(rep[:], recip[:])
                nc.vector.tensor_mul(
                    ctxT[m][po:po + HD, 512 * qc:512 * (qc + 1)],
                    ctx_ps[0:HD, :], rep[:])
                # sprinkle previous chunk's out-projection between heads
                if qc > 0 and h % 2 == 1:
                    emit_outproj(4 * (qc - 1) + h // 2)

        for qt in range(4 * (NQC - 1), 4 * NQC):
            emit_outproj(qt)

    nc.compile()
    return nc


def _shard(inputs):
    in_maps = []
    for c in range(N_CORES):
        b, g = c // 2, c % 2
        sl = slice(512 * g, 512 * (g + 1))
        in_maps.append({
            "xqt": np.ascontiguousarray(inputs["inputs_q"][b].T.astype(ml_dtypes.bfloat16)),
            "xkt": np.ascontiguousarray(inputs["inputs_k"][b].T.astype(ml_dtypes.bfloat16)),
            "xvt": np.ascontiguousarray(inputs["inputs_v"][b].T.astype(ml_dtypes.bfloat16)),
            "wq": np.ascontiguousarray(inputs["Wq"][:, sl].astype(ml_dtypes.bfloat16)),
            "wk": np.ascontiguousarray(inputs["Wk"][:, sl].astype(ml_dtypes.bfloat16)),
            "wv": np.ascontiguousarray(inputs["Wv"][:, sl].astype(ml_dtypes.bfloat16)),
            "bq": np.ascontiguousarray(inputs["bq"][sl])[:, None],
            "bk": np.ascontiguousarray(inputs["bk"][sl])[:, None],
            "bvb": np.ascontiguousarray(
                np.broadcast_to(inputs["bv"][sl], (128, 512))),
            "wo": np.ascontiguousarray(inputs["Wo"][sl, :].astype(ml_dtypes.bfloat16)),
        })
    return in_maps


def kernel(**inputs):
    global _compiled
    inputs = {k: np.asarray(v, dtype=np.float32) for k, v in inputs.items()}
    if _compiled is None:
        _compiled = _build()
    nc = _compiled
    in_maps = _shard(inputs)
    res = run_bass_kernel_spmd(nc, in_maps, list(range(N_CORES)),
                               trace=bool(int(__import__("os").environ.get("BASS_TRACE", "0"))))
    kernel.last_results = res
    B = 4
    out = np.empty((B, S, D), np.float32)
    for b in range(B):
        out[b] = res.results[2 * b]["out"] + res.results[2 * b + 1]["out"]
    out += inputs["bo"][None, None, :]
    return out
